# revision 28
# baseline (speedup 1.0000x reference)
"""Batch graph-attention (GAT) layer on 8 TRN2 NeuronCores - Bass/Tile kernel.

kernel(**inputs) takes the FULL inputs
  X [4,2048,64] f32, A [4,2048,2048] f32 (0/1 adjacency),
  W [4,64,64] f32, a_self [4,64] f32, a_neigh [4,64] f32
and returns the FULL output [4,2048,256] f32.

Sharding: data-parallel over (batch, query-half): core c handles batch c//2,
query rows [(c%2)*1024, (c%2)*1024+1024).  No collectives.

Host-side prep is layout-only: per-core slices, X^T / Xq^T transposes,
A slice cast to bf16 (exact for 0/1 adjacency), and the usual weight packing
[W_0..W_3 | W a_self | W a_neigh].  All math runs on device.

Math (per core, per head h), exploiting softmax scale-invariance:
  exp(lrelu_0.2(s1[i]+s2[j])) = exp(s1[i]) * max(E1[j], G[i]*E2[j])
  with E1=exp(s2), E2=exp(0.2*s2), G=exp(-0.8*s1); the exp(s1[i]) factor is
  constant per query column i and cancels in the softmax division, so the
  kernel never materializes it.  Each [128j x 1024i] score tile is then ONE
  dual-scalar TensorScalar on DVE (bf16, 4x mode):
      m = (G_bc * E2[j]) max E1[j]
  followed by one bf16 mask multiply pm = m * A^T (DVE 2x / Pool for a
  subset), and a bf16 feats matmul [lin|1]^T @ pm accumulating numerator +
  denominator in PSUM.  Division (+ ReLU) happens once per (head, query) on
  the Activation engine.

Implementation notes:
 - A^T comes from 16 DMA-xbar transposes ([1024,128] -> [128,1024]) reading
   the bf16 input directly; no staging, no conversion pass.
 - lin / score matmuls run in float32r (1 cycle/row vs 4 for fp32); f32r
   requires producers to round, so X^T / W go through f32r-rounding copies.
 - feats matmuls run in bf16; PSUM accumulates fp32.  Main loop iterates
   j-tile pairs outer / heads inner with 4 concurrent PSUM accumulators, so
   compute consumes A^T tiles in xbar completion order.
 - DMA queues: bulk loads + xbars + output on the SP queue; the
   compute-dependent DMAs (sq scratch write, s_self broadcasts) go on the
   Activation queue so they cannot head-of-line-block the xbars.
 - This walrus build accepts at most one sync-wait per instruction; a
   post-scheduling pass splits Tile's multi-wait instructions into wait-only
   EventSemaphore sequencer ops (engine queues are strict FIFO).
"""
import sys

if "/opt/trn_rl_repo" not in sys.path:
    sys.path.insert(0, "/opt/trn_rl_repo")

import numpy as np
import ml_dtypes
import concourse.bass as bass
import concourse.tile as tile
from concourse import mybir
from concourse.bass_utils import run_bass_kernel_spmd

F32 = mybir.dt.float32
F32R = mybir.dt.float32r
BF16 = mybir.dt.bfloat16

B, N, F, H, FE = 4, 2048, 64, 4, 64
NI = 1024
NT = N // 128
NIC = NI // 128
ALPHA = 0.2
LW = FE + 1
LEXT = H * LW
# (pr*H + h) indices (of 32) whose mask-multiply runs on Pool
POOL_PAIRS = frozenset(
    [pr * H + 3 for pr in range(7)] + [pr * H + 1 for pr in (2, 3, 4)])


def _split_multi_waits(nc, max_waits=1):
    """Split multi-wait instructions (walrus limit: 1 sync-wait per inst)."""
    n_split = 0
    for fn in nc.m.functions:
        for blk in fn.blocks:
            insts = blk.instructions
            i = 0
            while i < len(insts):
                inst = insts[i]
                si = inst.sync_info
                if si is None or len(si.on_wait) <= max_waits:
                    i += 1
                    continue
                waits = list(si.on_wait)
                extra, keep = waits[:-max_waits], waits[-max_waits:]
                for w in extra:
                    ev = mybir.InstEventSemaphore(
                        name=f"{inst.name}_wsplit{n_split}", ins=[], outs=[])
                    ev.engine = inst.engine
                    ev.sync_info = mybir.SyncInfo(on_wait=[w], on_update=[])
                    insts.insert(i, ev)
                    n_split += 1
                    i += 1
                inst.sync_info = mybir.SyncInfo(
                    on_wait=keep, on_update=list(si.on_update))
                i += 1
    return n_split


def _emit(tc, outs, ins, reps=1, hw_loop=False):
    if hw_loop and reps > 1:
        with tc.For_i(0, reps, 1,
                      hint_engines=(mybir.EngineType.PE, mybir.EngineType.DVE,
                                    mybir.EngineType.Activation,
                                    mybir.EngineType.SP,
                                    mybir.EngineType.Pool)):
            _emit_once(tc, outs, ins, 0)
    else:
        for rep in range(reps):
            _emit_once(tc, outs, ins, rep)


def _emit_once(tc, outs, ins, rep):
    """Emit the kernel into an open TileContext."""
    nc = tc.nc
    outD = outs[0] if isinstance(outs, (list, tuple)) else outs
    XTD, AhD, WallD = ins

    const = tc.alloc_tile_pool(name="const", bufs=1)
    persist = tc.alloc_tile_pool(name="persist", bufs=1)
    work = tc.alloc_tile_pool(name="work", bufs=8)
    gwork = tc.alloc_tile_pool(name="gwork", bufs=2)
    outw = tc.alloc_tile_pool(name="outw", bufs=2)
    ps_small = tc.alloc_tile_pool(name="ps_small", bufs=2, space="PSUM")

    # ---- constants / inputs: two packed loads (SP queue) ----
    # WI = [Wall (264 cols, rows 0..63) | Identity (128) | head-selector
    #       one-hot rows (4 x 128 cols, rows 0..3)]
    WI_sb = const.tile([128, LEXT + 4 + 128 + 512], F32)
    nc.sync.dma_start(out=WI_sb, in_=WallD)
    W_sb = WI_sb[0:F, 0:LEXT + 4]
    I_sb = WI_sb[:, LEXT + 4:LEXT + 4 + 128]
    sel_bf = const.tile([H, 512], BF16)
    nc.vector.tensor_copy(sel_bf, WI_sb[0:H, LEXT + 4 + 128:LEXT + 4 + 640])
    # XTT = [XqT (1024 cols) | XT (2048 cols)], rows 0..63
    XTT_sb = persist.tile([F, NI + N], F32)
    nc.sync.dma_start(out=XTT_sb, in_=XTD)
    XqT_sb = XTT_sb[:, 0:NI]
    XT_sb = XTT_sb[:, NI:NI + N]

    # ---- A^T from the bf16 input: 8 pair-granular xbar transposes (SP);
    # 3D out [128, 2, 1024] maps source column s*128+p, row r -> AT[p, s, r]
    AT_p = [persist.tile([128, 2 * NI], BF16, tag=f"ATp{k}", name=f"ATp{k}")
            for k in range(NT // 2)]
    for k in range(NT // 2):
        nc.sync.dma_start_transpose(
            out=AT_p[k].rearrange("p (s f) -> p s f", s=2),
            in_=AhD[:, k * 256:(k + 1) * 256])

    # ---- f32r-rounding copies for the score matmuls ----
    W_r = const.tile([F, LEXT + 4], F32R)
    nc.vector.tensor_copy(W_r, W_sb)
    XqT_r = persist.tile([F, NI], F32R)
    nc.vector.tensor_copy(XqT_r, XqT_sb)
    XT_r = persist.tile([F, N], F32R)
    nc.vector.tensor_copy(XT_r, XT_sb)

    # ---- G = exp(-0.8 * s_self) for this core's queries ----
    # One transposed matmul puts per-head query scores on partitions 0..3:
    # s1T[h, i] = (W a_self)_h . Xq_i; then one exp, one 2KB DRAM write and
    # four contiguous broadcast reads produce the G_bc tiles.
    s1T_ps = ps_small.tile([H, NI], F32, tag="s1T", bufs=1)
    for half in range(2):
        nc.tensor.matmul(
            out=s1T_ps[:, half * 512:(half + 1) * 512],
            lhsT=W_r[:, H * FE:H * FE + H],
            rhs=XqT_r[:, half * 512:(half + 1) * 512],
            start=True, stop=True)
    G_all = persist.tile([H, NI], BF16)
    nc.scalar.activation(out=G_all, in_=s1T_ps, scale=ALPHA - 1.0,
                         func=mybir.ActivationFunctionType.Exp)

    # ---- lin (bf16, for feats matmuls) + neighbor scores s2 ----
    # Separate per-tile tensors so consumers unblock as each tile lands
    # (whole-tile RAW tracking would otherwise chain them all).
    linext = [persist.tile([128, LEXT], BF16, tag=f"lx{t}", name=f"lx{t}")
              for t in range(NT)]
    E1s = [persist.tile([128, 8], F32, tag=f"E1_{t}", name=f"E1_{t}")
           for t in range(NT)]
    E2s = [persist.tile([128, 8], F32, tag=f"E2_{t}", name=f"E2_{t}")
           for t in range(NT)]

    def emit_lin(ts):
        for t in ts:
            lin3 = linext[t].rearrange("p (h c) -> p h c", h=H)
            nc.vector.memset(lin3[:, :, FE:FE + 1], 1.0)
            lin_ps = ps_small.tile([128, LEXT + 4], F32, tag="linps", bufs=2,
                                   name=f"lin_ps{t}")
            nc.tensor.matmul(
                out=lin_ps, lhsT=XT_r[:, t * 128:(t + 1) * 128], rhs=W_r,
                start=True, stop=True)
            nc.scalar.activation(out=E1s[t], in_=lin_ps[:, H * FE:H * FE + 8],
                                 func=mybir.ActivationFunctionType.Exp)
            nc.scalar.activation(out=E2s[t], in_=lin_ps[:, H * FE:H * FE + 8],
                                 scale=ALPHA,
                                 func=mybir.ActivationFunctionType.Exp)
            nc.scalar.copy(
                lin3[:, :, 0:FE],
                lin_ps[:, 0:H * FE].rearrange("p (h o) -> p h o", h=H))

    emit_lin(range(4))

    # G_bc via PE selector broadcasts: sel_h[4,128] (one-hot row h) against
    # G_all[4,NI] -> PSUM [128,NI], copied to bf16 SBUF on Act.  No DMA.
    G_bc = []
    for h in range(H):
        g_ps = ps_small.tile([128, NI], F32, tag="gps", bufs=2,
                             name=f"g_ps{h}")
        for half in range(2):
            nc.tensor.matmul(
                out=g_ps[:, half * 512:(half + 1) * 512],
                lhsT=sel_bf[:, h * 128:(h + 1) * 128],
                rhs=G_all[:, half * 512:(half + 1) * 512],
                start=True, stop=True)
        g = gwork.tile([128, NI], BF16, tag=f"G{h}", name=f"G{h}")
        nc.scalar.copy(g, g_ps)
        G_bc.append(g)
    emit_lin(range(4, NT))

    ps_small.release()
    ps_feats = tc.alloc_tile_pool(name="ps_feats", bufs=1, space="PSUM")

    # ---- main loop: pairs of j-tiles outer (consumes A^T in xbar
    # completion order), heads inner; 4 concurrent PSUM accumulators ----
    feats_ps = [ps_feats.tile([LW, NI], F32, tag=f"feats{h}",
                              name=f"feats{h}") for h in range(H)]
    NPR = NT // 2
    for pr in range(NPR):
        for h in range(H):
            m2 = work.tile([128, 2 * NI], BF16, tag="m2")
            for k in range(2):
                jt = pr * 2 + k
                nc.vector.tensor_scalar(
                    out=m2[:, k * NI:(k + 1) * NI], in0=G_bc[h],
                    scalar1=E2s[jt][:, H + h:H + h + 1],
                    scalar2=E1s[jt][:, H + h:H + h + 1],
                    op0=mybir.AluOpType.mult, op1=mybir.AluOpType.max)
            pm2 = work.tile([128, 2 * NI], BF16, tag="pm2")
            eng = nc.gpsimd if (pr * H + h) in POOL_PAIRS else nc.vector
            eng.tensor_mul(pm2, m2, AT_p[pr])
            for k in range(2):
                for half in range(2):
                    jt = pr * 2 + k
                    nc.tensor.matmul(
                        out=feats_ps[h][:, half * 512:(half + 1) * 512],
                        lhsT=linext[jt][:, h * LW:(h + 1) * LW],
                        rhs=pm2[:, k * NI + half * 512:
                                k * NI + (half + 1) * 512],
                        start=(pr == 0 and k == 0),
                        stop=(pr == NPR - 1 and k == 1))

    # ---- output stage ----
    out_sb = persist.tile([128, NIC * H * FE], F32)
    feats_sb = [outw.tile([LW, NI], F32, tag=f"featsb{h}", name=f"featsb{h}")
                for h in range(H)]
    for h in range(H):
        nc.scalar.copy(feats_sb[h], feats_ps[h])
    ps_feats.release()
    ps_outT = tc.alloc_tile_pool(name="ps_outT", bufs=2, space="PSUM")
    for h in range(H):
        # one [65,128]->[128,65] transpose per query block carries the
        # numerators AND the denominator column; two PSUM tiles of 4 blocks
        # each so no transpose output straddles a PSUM bank boundary
        fT_ps = [ps_outT.tile([128, 4 * LW], F32, tag=f"fT{half}",
                              name=f"fT{half}") for half in range(2)]
        for ic in range(NIC):
            nc.tensor.transpose(
                out=fT_ps[ic // 4][:, (ic % 4) * LW:(ic % 4 + 1) * LW],
                in_=feats_sb[h][:, ic * 128:(ic + 1) * 128],
                identity=I_sb[0:LW, 0:LW])
        recips = outw.tile([128, NIC], F32, tag="recips")
        for half in range(2):
            nc.vector.reciprocal(
                recips[:, half * 4:(half + 1) * 4],
                fT_ps[half].rearrange("p (q c) -> p q c", q=4)[:, :, FE])
        for ic in range(NIC):
            # out = relu(feats / denom), alternating DVE / Act per head
            if h % 2 == 0:
                nc.vector.tensor_scalar(
                    out=out_sb[:, ic * H * FE + h * FE:
                               ic * H * FE + (h + 1) * FE],
                    in0=fT_ps[ic // 4][:, (ic % 4) * LW:(ic % 4) * LW + FE],
                    scalar1=recips[:, ic:ic + 1], scalar2=0.0,
                    op0=mybir.AluOpType.mult, op1=mybir.AluOpType.max)
            else:
                nc.scalar.activation(
                    out=out_sb[:, ic * H * FE + h * FE:
                               ic * H * FE + (h + 1) * FE],
                    in_=fT_ps[ic // 4][:, (ic % 4) * LW:(ic % 4) * LW + FE],
                    scale=recips[:, ic:ic + 1],
                    func=mybir.ActivationFunctionType.Relu)

    nc.sync.dma_start(
        out=outD.rearrange("(t p) o -> p t o", p=128),
        in_=out_sb.rearrange("p (t o) -> p t o", t=NIC))

    for p in (ps_outT, outw, gwork, work, persist, const):
        p.release()


_CACHED = {}


def _build_nc(reps=1, hw_loop=False):
    key = (reps, hw_loop)
    if key in _CACHED:
        return _CACHED[key]
    nc = bass.Bass("TRN2", target_bir_lowering=False, debug=False,
                   num_devices=8)
    xtt = nc.dram_tensor("XTT", [F, NI + N], F32, kind="ExternalInput").ap()
    ah = nc.dram_tensor("Ah", [NI, N], BF16, kind="ExternalInput").ap()
    wi = nc.dram_tensor("WI", [128, LEXT + 4 + 128 + 512], F32,
                        kind="ExternalInput").ap()
    out = nc.dram_tensor("Out", [NI, H * FE], F32, kind="ExternalOutput").ap()
    with tile.TileContext(nc) as tc:
        _emit(tc, [out], [xtt, ah, wi], reps=reps, hw_loop=hw_loop)
    _split_multi_waits(nc)
    _CACHED[key] = nc
    return nc


def _make_in_maps(X, A, W, a_self, a_neigh):
    C2self = np.einsum("hfo,ho->fh", W, a_self)
    C2neigh = np.einsum("hfo,ho->fh", W, a_neigh)
    Wall = np.concatenate(
        [W[h] for h in range(H)] + [C2self, C2neigh], axis=1)
    WI = np.zeros((128, LEXT + 4 + 128 + 512), np.float32)
    WI[0:F, 0:LEXT + 4] = Wall
    WI[:, LEXT + 4:LEXT + 4 + 128] = np.eye(128, dtype=np.float32)
    for h in range(H):
        WI[h, LEXT + 4 + 128 + h * 128:LEXT + 4 + 128 + (h + 1) * 128] = 1.0
    in_maps = []
    for c in range(8):
        b, ih = c // 2, c % 2
        i0 = ih * NI
        XTT = np.concatenate(
            [X[b, i0:i0 + NI].T, X[b].T], axis=1).astype(np.float32)
        in_maps.append({
            "XTT": np.ascontiguousarray(XTT),
            "Ah": np.ascontiguousarray(
                A[b, i0:i0 + NI, :]).astype(ml_dtypes.bfloat16),
            "WI": WI,
        })
    return in_maps


def kernel(X, A, W, a_self, a_neigh):
    X = np.asarray(X, np.float32)
    A = np.asarray(A, np.float32)
    W = np.asarray(W, np.float32)
    a_self = np.asarray(a_self, np.float32)
    a_neigh = np.asarray(a_neigh, np.float32)
    in_maps = _make_in_maps(X, A, W, a_self, a_neigh)
    nc = _build_nc()
    res = run_bass_kernel_spmd(nc, in_maps, list(range(8)))
    out = np.empty((B, N, H * FE), np.float32)
    for c in range(8):
        b, ih = c // 2, c % 2
        out[b, ih * NI:(ih + 1) * NI, :] = res.results[c]["Out"]
    return out


def measure_exec_ns(inputs, loop_reps=512, calls=8):
    """Differential device-time measurement: wrap the kernel body in an
    on-device For_i loop with `loop_reps` iterations; with device-resident
    inputs, exec_ns = (min_wall(loop) - min_wall(single)) / (loop_reps - 1).
    Each iteration re-reads all inputs from HBM (full single-shot kernel,
    with a full inter-iteration barrier at the loop back-edge)."""
    import time as _time
    import jax
    from jax.sharding import Mesh, PartitionSpec, NamedSharding
    from jax.experimental.shard_map import shard_map
    from concourse.bass2jax import (_bass_exec_p, install_neuronx_cc_hook,
                                    partition_id_tensor)

    in_maps = _make_in_maps(
        np.asarray(inputs["X"], np.float32), np.asarray(inputs["A"], np.float32),
        np.asarray(inputs["W"], np.float32),
        np.asarray(inputs["a_self"], np.float32),
        np.asarray(inputs["a_neigh"], np.float32))

    def runner(nc, n_cores=8):
        install_neuronx_cc_hook()
        in_names, out_names, out_avals, zero_outs = [], [], [], []
        for alloc in nc.m.functions[0].allocations:
            if not isinstance(alloc, mybir.MemoryLocationSet):
                continue
            name = alloc.memorylocations[0].name
            if alloc.kind == "ExternalInput":
                in_names.append(name)
            elif alloc.kind == "ExternalOutput":
                out_names.append(name)
                shape = tuple(alloc.tensor_shape)
                dtype = mybir.dt.np(alloc.dtype)
                out_avals.append(jax.core.ShapedArray(shape, dtype))
                zero_outs.append(np.zeros(shape, dtype))
        pname = nc.partition_id_tensor.name if nc.partition_id_tensor else None
        if pname in in_names:
            in_names.remove(pname)
        n_params = len(in_names)
        all_in = in_names + out_names + ([pname] if pname else [])

        def _body(*args):
            ops = list(args)
            if pname:
                ops.append(partition_id_tensor())
            return tuple(_bass_exec_p.bind(
                *ops, out_avals=tuple(out_avals), in_names=tuple(all_in),
                out_names=tuple(out_names), lowering_input_output_aliases=(),
                sim_require_finite=True, sim_require_nnan=True, nc=nc))

        devices = jax.devices()[:n_cores]
        mesh = Mesh(np.asarray(devices), ("core",))
        nio = n_params + len(out_names)
        fn = jax.jit(shard_map(_body, mesh=mesh,
                               in_specs=(PartitionSpec("core"),) * nio,
                               out_specs=(PartitionSpec("core"),) * len(out_names),
                               check_rep=False), keep_unused=True)
        sh = NamedSharding(mesh, PartitionSpec("core"))
        cin = [jax.device_put(np.concatenate(
                   [np.asarray(in_maps[c][nm]) for c in range(n_cores)], axis=0),
                   sh) for nm in in_names]
        czs = [jax.device_put(
                   np.zeros((n_cores * z.shape[0], *z.shape[1:]), z.dtype), sh)
               for z in zero_outs]
        jax.block_until_ready(cin + czs)

        def run():
            jax.block_until_ready(fn(*cin, *czs))
        return run

    mins = {}
    for reps in (1, loop_reps):
        run = runner(_build_nc(reps, hw_loop=(reps > 1)))
        run()
        walls = []
        for _ in range(calls):
            t0 = _time.time()
            run()
            walls.append(_time.time() - t0)
        mins[reps] = min(walls)
    return (mins[loop_reps] - mins[1]) / (loop_reps - 1) * 1e9


# revision 29
# speedup vs baseline: 1.1590x; 1.1590x over previous
"""Batch graph-attention (GAT) layer on 8 TRN2 NeuronCores - Bass/Tile kernel.

kernel(**inputs) takes the FULL inputs
  X [4,2048,64] f32, A [4,2048,2048] f32 (0/1 adjacency),
  W [4,64,64] f32, a_self [4,64] f32, a_neigh [4,64] f32
and returns the FULL output [4,2048,256] f32.

Sharding: data-parallel over (batch, query-half): core c handles batch c//2,
query rows [(c%2)*1024, (c%2)*1024+1024).  No collectives.

Host-side prep is layout-only: per-core slices, X^T / Xq^T transposes,
A slice cast to bf16 (exact for 0/1 adjacency), and the usual weight packing
[W_0..W_3 | W a_self | W a_neigh].  All math runs on device.

Math (per core, per head h), exploiting softmax scale-invariance:
  exp(lrelu_0.2(s1[i]+s2[j])) = exp(s1[i]) * max(E1[j], G[i]*E2[j])
  with E1=exp(s2), E2=exp(0.2*s2), G=exp(-0.8*s1); the exp(s1[i]) factor is
  constant per query column i and cancels in the softmax division, so the
  kernel never materializes it.  Each [128j x 1024i] score tile is then ONE
  dual-scalar TensorScalar on DVE (bf16, 4x mode):
      m = (G_bc * E2[j]) max E1[j]
  followed by one bf16 mask multiply pm = m * A^T (DVE 2x / Pool for a
  subset), and a bf16 feats matmul [lin|1]^T @ pm accumulating numerator +
  denominator in PSUM.  Division (+ ReLU) happens once per (head, query) on
  the Activation engine.

Implementation notes:
 - A^T comes from 16 DMA-xbar transposes ([1024,128] -> [128,1024]) reading
   the bf16 input directly; no staging, no conversion pass.
 - lin / score matmuls run in float32r (1 cycle/row vs 4 for fp32); f32r
   requires producers to round, so X^T / W go through f32r-rounding copies.
 - feats matmuls run in bf16; PSUM accumulates fp32.  Main loop iterates
   j-tile pairs outer / heads inner with 4 concurrent PSUM accumulators, so
   compute consumes A^T tiles in xbar completion order.
 - DMA queues: bulk loads + xbars + output on the SP queue; the
   compute-dependent DMAs (sq scratch write, s_self broadcasts) go on the
   Activation queue so they cannot head-of-line-block the xbars.
 - This walrus build accepts at most one sync-wait per instruction; a
   post-scheduling pass splits Tile's multi-wait instructions into wait-only
   EventSemaphore sequencer ops (engine queues are strict FIFO).
"""
import sys

if "/opt/trn_rl_repo" not in sys.path:
    sys.path.insert(0, "/opt/trn_rl_repo")

import numpy as np
import ml_dtypes
import concourse.bass as bass
import concourse.tile as tile
from concourse import mybir
from concourse.bass_utils import run_bass_kernel_spmd

F32 = mybir.dt.float32
F32R = mybir.dt.float32r
BF16 = mybir.dt.bfloat16

B, N, F, H, FE = 4, 2048, 64, 4, 64
NI = 1024
NT = N // 128
NIC = NI // 128
ALPHA = 0.2
LW = FE + 1
LEXT = H * LW
# (pr*H + h) indices (of 32) whose mask-multiply runs on Pool
POOL_PAIRS = frozenset(
    [pr * H + 3 for pr in range(7)] + [pr * H + 1 for pr in (2, 3, 4)])


def _split_multi_waits(nc, max_waits=1):
    """Split multi-wait instructions (walrus limit: 1 sync-wait per inst)."""
    n_split = 0
    for fn in nc.m.functions:
        for blk in fn.blocks:
            insts = blk.instructions
            i = 0
            while i < len(insts):
                inst = insts[i]
                si = inst.sync_info
                if si is None or len(si.on_wait) <= max_waits:
                    i += 1
                    continue
                waits = list(si.on_wait)
                extra, keep = waits[:-max_waits], waits[-max_waits:]
                for w in extra:
                    ev = mybir.InstEventSemaphore(
                        name=f"{inst.name}_wsplit{n_split}", ins=[], outs=[])
                    ev.engine = inst.engine
                    ev.sync_info = mybir.SyncInfo(on_wait=[w], on_update=[])
                    insts.insert(i, ev)
                    n_split += 1
                    i += 1
                inst.sync_info = mybir.SyncInfo(
                    on_wait=keep, on_update=list(si.on_update))
                i += 1
    return n_split


def _emit(tc, outs, ins, reps=1, hw_loop=False):
    if hw_loop and reps > 1:
        with tc.For_i(0, reps, 1,
                      hint_engines=(mybir.EngineType.PE, mybir.EngineType.DVE,
                                    mybir.EngineType.Activation,
                                    mybir.EngineType.SP,
                                    mybir.EngineType.Pool)):
            _emit_once(tc, outs, ins, 0)
    else:
        for rep in range(reps):
            _emit_once(tc, outs, ins, rep)


def _emit_once(tc, outs, ins, rep):
    """Emit the kernel into an open TileContext."""
    nc = tc.nc
    outD = outs[0] if isinstance(outs, (list, tuple)) else outs
    XTD, AhD, WallD = ins

    const = tc.alloc_tile_pool(name="const", bufs=1)
    persist = tc.alloc_tile_pool(name="persist", bufs=1)
    work = tc.alloc_tile_pool(name="work", bufs=4)
    gwork = tc.alloc_tile_pool(name="gwork", bufs=2)
    outw = tc.alloc_tile_pool(name="outw", bufs=2)
    ps_small = tc.alloc_tile_pool(name="ps_small", bufs=2, space="PSUM")

    # ---- constants / inputs: two packed loads (SP queue) ----
    # WI = [Wall (264 cols, rows 0..63) | Identity (128) | head-selector
    #       one-hot rows (4 x 128 cols, rows 0..3)]
    WI_sb = const.tile([128, LEXT + 4 + 128 + 512], F32)
    nc.sync.dma_start(out=WI_sb, in_=WallD)
    W_sb = WI_sb[0:F, 0:LEXT + 4]
    I_sb = WI_sb[:, LEXT + 4:LEXT + 4 + 128]
    sel_bf = const.tile([H, 512], BF16)
    nc.vector.tensor_copy(sel_bf, WI_sb[0:H, LEXT + 4 + 128:LEXT + 4 + 640])
    # XTT = [XqT (1024 cols) | XT (2048 cols)], rows 0..63
    XTT_sb = persist.tile([F, NI + N], F32)
    nc.sync.dma_start(out=XTT_sb, in_=XTD)
    XqT_sb = XTT_sb[:, 0:NI]
    XT_sb = XTT_sb[:, NI:NI + N]

    # ---- A^T from the bf16 input: 8 pair-granular xbar transposes (SP);
    # 3D out [128, 2, 1024] maps source column s*128+p, row r -> AT[p, s, r]
    AT_p = [persist.tile([128, 2 * NI], BF16, tag=f"ATp{k}", name=f"ATp{k}")
            for k in range(NT // 2)]
    for k in range(NT // 2):
        nc.sync.dma_start_transpose(
            out=AT_p[k].rearrange("p (s f) -> p s f", s=2),
            in_=AhD[:, k * 256:(k + 1) * 256])

    # ---- f32r-rounding copies for the score matmuls ----
    W_r = const.tile([F, LEXT + 4], F32R)
    nc.vector.tensor_copy(W_r, W_sb)
    XqT_r = persist.tile([F, NI], F32R)
    nc.vector.tensor_copy(XqT_r, XqT_sb)
    XT_r = persist.tile([F, N], F32R)
    nc.vector.tensor_copy(XT_r, XT_sb)

    # ---- G = exp(-0.8 * s_self) for this core's queries ----
    # One transposed matmul puts per-head query scores on partitions 0..3:
    # s1T[h, i] = (W a_self)_h . Xq_i; then one exp, one 2KB DRAM write and
    # four contiguous broadcast reads produce the G_bc tiles.
    s1T_ps = ps_small.tile([H, NI], F32, tag="s1T", bufs=1)
    for half in range(2):
        nc.tensor.matmul(
            out=s1T_ps[:, half * 512:(half + 1) * 512],
            lhsT=W_r[:, H * FE:H * FE + H],
            rhs=XqT_r[:, half * 512:(half + 1) * 512],
            start=True, stop=True)
    G_all = persist.tile([H, NI], BF16)
    nc.scalar.activation(out=G_all, in_=s1T_ps, scale=ALPHA - 1.0,
                         func=mybir.ActivationFunctionType.Exp)

    # ---- lin (bf16, for feats matmuls) + neighbor scores s2 ----
    # Separate per-tile tensors so consumers unblock as each tile lands
    # (whole-tile RAW tracking would otherwise chain them all).
    linext = [persist.tile([128, LEXT], BF16, tag=f"lx{t}", name=f"lx{t}")
              for t in range(NT)]
    E1s = [persist.tile([128, 8], F32, tag=f"E1_{t}", name=f"E1_{t}")
           for t in range(NT)]
    E2s = [persist.tile([128, 8], F32, tag=f"E2_{t}", name=f"E2_{t}")
           for t in range(NT)]

    def emit_lin(ts):
        for t in ts:
            lin3 = linext[t].rearrange("p (h c) -> p h c", h=H)
            nc.vector.memset(lin3[:, :, FE:FE + 1], 1.0)
            lin_ps = ps_small.tile([128, LEXT + 4], F32, tag="linps", bufs=2,
                                   name=f"lin_ps{t}")
            nc.tensor.matmul(
                out=lin_ps, lhsT=XT_r[:, t * 128:(t + 1) * 128], rhs=W_r,
                start=True, stop=True)
            nc.scalar.activation(out=E1s[t], in_=lin_ps[:, H * FE:H * FE + 8],
                                 func=mybir.ActivationFunctionType.Exp)
            nc.scalar.activation(out=E2s[t], in_=lin_ps[:, H * FE:H * FE + 8],
                                 scale=ALPHA,
                                 func=mybir.ActivationFunctionType.Exp)
            nc.scalar.copy(
                lin3[:, :, 0:FE],
                lin_ps[:, 0:H * FE].rearrange("p (h o) -> p h o", h=H))

    emit_lin(range(4))

    # G_bc via PE selector broadcasts: sel_h[4,128] (one-hot row h) against
    # G_all[4,NI] -> PSUM [128,NI], copied to bf16 SBUF on Act.  No DMA.
    G_bc = []
    for h in range(H):
        g_ps = ps_small.tile([128, NI], F32, tag="gps", bufs=2,
                             name=f"g_ps{h}")
        for half in range(2):
            nc.tensor.matmul(
                out=g_ps[:, half * 512:(half + 1) * 512],
                lhsT=sel_bf[:, h * 128:(h + 1) * 128],
                rhs=G_all[:, half * 512:(half + 1) * 512],
                start=True, stop=True)
        g = gwork.tile([128, NI], BF16, tag=f"G{h}", name=f"G{h}")
        nc.scalar.copy(g, g_ps)
        G_bc.append(g)
    emit_lin(range(4, NT))

    ps_small.release()
    ps_feats = tc.alloc_tile_pool(name="ps_feats", bufs=1, space="PSUM")

    # ---- main loop: pairs of j-tiles outer (consumes A^T in xbar
    # completion order), heads inner; 4 concurrent PSUM accumulators ----
    feats_ps = [ps_feats.tile([LW, NI], F32, tag=f"feats{h}",
                              name=f"feats{h}") for h in range(H)]
    NPR = NT // 2
    for pr in range(NPR):
        for h in range(H):
            m2 = work.tile([128, 2 * NI], BF16, tag="m2")
            for k in range(2):
                jt = pr * 2 + k
                nc.vector.tensor_scalar(
                    out=m2[:, k * NI:(k + 1) * NI], in0=G_bc[h],
                    scalar1=E2s[jt][:, H + h:H + h + 1],
                    scalar2=E1s[jt][:, H + h:H + h + 1],
                    op0=mybir.AluOpType.mult, op1=mybir.AluOpType.max)
            pm2 = work.tile([128, 2 * NI], BF16, tag="pm2")
            eng = nc.gpsimd if (pr * H + h) in POOL_PAIRS else nc.vector
            eng.tensor_mul(pm2, m2, AT_p[pr])
            for k in range(2):
                for half in range(2):
                    jt = pr * 2 + k
                    nc.tensor.matmul(
                        out=feats_ps[h][:, half * 512:(half + 1) * 512],
                        lhsT=linext[jt][:, h * LW:(h + 1) * LW],
                        rhs=pm2[:, k * NI + half * 512:
                                k * NI + (half + 1) * 512],
                        start=(pr == 0 and k == 0),
                        stop=(pr == NPR - 1 and k == 1))

    # ---- output stage ----
    out_sb = persist.tile([128, NIC * H * FE], F32)
    feats_sb = [outw.tile([LW, NI], F32, tag=f"featsb{h}", name=f"featsb{h}")
                for h in range(H)]
    for h in range(H):
        nc.scalar.copy(feats_sb[h], feats_ps[h])
    ps_feats.release()
    ps_outT = tc.alloc_tile_pool(name="ps_outT", bufs=2, space="PSUM")
    for h in range(H):
        # one [65,128]->[128,65] transpose per query block carries the
        # numerators AND the denominator column; two PSUM tiles of 4 blocks
        # each so no transpose output straddles a PSUM bank boundary
        fT_ps = [ps_outT.tile([128, 4 * LW], F32, tag=f"fT{half}",
                              name=f"fT{half}") for half in range(2)]
        for ic in range(NIC):
            nc.tensor.transpose(
                out=fT_ps[ic // 4][:, (ic % 4) * LW:(ic % 4 + 1) * LW],
                in_=feats_sb[h][:, ic * 128:(ic + 1) * 128],
                identity=I_sb[0:LW, 0:LW])
        recips = outw.tile([128, NIC], F32, tag="recips")
        for half in range(2):
            nc.vector.reciprocal(
                recips[:, half * 4:(half + 1) * 4],
                fT_ps[half].rearrange("p (q c) -> p q c", q=4)[:, :, FE])
        for ic in range(NIC):
            # out = relu(feats / denom), alternating DVE / Act per head
            if h % 2 == 0:
                nc.vector.tensor_scalar(
                    out=out_sb[:, ic * H * FE + h * FE:
                               ic * H * FE + (h + 1) * FE],
                    in0=fT_ps[ic // 4][:, (ic % 4) * LW:(ic % 4) * LW + FE],
                    scalar1=recips[:, ic:ic + 1], scalar2=0.0,
                    op0=mybir.AluOpType.mult, op1=mybir.AluOpType.max)
            else:
                nc.scalar.activation(
                    out=out_sb[:, ic * H * FE + h * FE:
                               ic * H * FE + (h + 1) * FE],
                    in_=fT_ps[ic // 4][:, (ic % 4) * LW:(ic % 4) * LW + FE],
                    scale=recips[:, ic:ic + 1],
                    func=mybir.ActivationFunctionType.Relu)

    nc.sync.dma_start(
        out=outD.rearrange("(t p) o -> p t o", p=128),
        in_=out_sb.rearrange("p (t o) -> p t o", t=NIC))

    for p in (ps_outT, outw, gwork, work, persist, const):
        p.release()


_CACHED = {}


def _build_nc(reps=1, hw_loop=False):
    key = (reps, hw_loop)
    if key in _CACHED:
        return _CACHED[key]
    nc = bass.Bass("TRN2", target_bir_lowering=False, debug=False,
                   num_devices=8)
    xtt = nc.dram_tensor("XTT", [F, NI + N], F32, kind="ExternalInput").ap()
    ah = nc.dram_tensor("Ah", [NI, N], BF16, kind="ExternalInput").ap()
    wi = nc.dram_tensor("WI", [128, LEXT + 4 + 128 + 512], F32,
                        kind="ExternalInput").ap()
    out = nc.dram_tensor("Out", [NI, H * FE], F32, kind="ExternalOutput").ap()
    with tile.TileContext(nc) as tc:
        _emit(tc, [out], [xtt, ah, wi], reps=reps, hw_loop=hw_loop)
    _split_multi_waits(nc)
    _CACHED[key] = nc
    return nc


def _make_in_maps(X, A, W, a_self, a_neigh):
    C2self = np.einsum("hfo,ho->fh", W, a_self)
    C2neigh = np.einsum("hfo,ho->fh", W, a_neigh)
    Wall = np.concatenate(
        [W[h] for h in range(H)] + [C2self, C2neigh], axis=1)
    WI = np.zeros((128, LEXT + 4 + 128 + 512), np.float32)
    WI[0:F, 0:LEXT + 4] = Wall
    WI[:, LEXT + 4:LEXT + 4 + 128] = np.eye(128, dtype=np.float32)
    for h in range(H):
        WI[h, LEXT + 4 + 128 + h * 128:LEXT + 4 + 128 + (h + 1) * 128] = 1.0
    in_maps = []
    for c in range(8):
        b, ih = c // 2, c % 2
        i0 = ih * NI
        XTT = np.concatenate(
            [X[b, i0:i0 + NI].T, X[b].T], axis=1).astype(np.float32)
        in_maps.append({
            "XTT": np.ascontiguousarray(XTT),
            "Ah": np.ascontiguousarray(
                A[b, i0:i0 + NI, :]).astype(ml_dtypes.bfloat16),
            "WI": WI,
        })
    return in_maps


def kernel(X, A, W, a_self, a_neigh):
    X = np.asarray(X, np.float32)
    A = np.asarray(A, np.float32)
    W = np.asarray(W, np.float32)
    a_self = np.asarray(a_self, np.float32)
    a_neigh = np.asarray(a_neigh, np.float32)
    in_maps = _make_in_maps(X, A, W, a_self, a_neigh)
    nc = _build_nc()
    res = run_bass_kernel_spmd(nc, in_maps, list(range(8)))
    out = np.empty((B, N, H * FE), np.float32)
    for c in range(8):
        b, ih = c // 2, c % 2
        out[b, ih * NI:(ih + 1) * NI, :] = res.results[c]["Out"]
    return out


def measure_exec_ns(inputs, loop_reps=512, calls=8):
    """Differential device-time measurement: wrap the kernel body in an
    on-device For_i loop with `loop_reps` iterations; with device-resident
    inputs, exec_ns = (min_wall(loop) - min_wall(single)) / (loop_reps - 1).
    Each iteration re-reads all inputs from HBM (full single-shot kernel,
    with a full inter-iteration barrier at the loop back-edge)."""
    import time as _time
    import jax
    from jax.sharding import Mesh, PartitionSpec, NamedSharding
    from jax.experimental.shard_map import shard_map
    from concourse.bass2jax import (_bass_exec_p, install_neuronx_cc_hook,
                                    partition_id_tensor)

    in_maps = _make_in_maps(
        np.asarray(inputs["X"], np.float32), np.asarray(inputs["A"], np.float32),
        np.asarray(inputs["W"], np.float32),
        np.asarray(inputs["a_self"], np.float32),
        np.asarray(inputs["a_neigh"], np.float32))

    def runner(nc, n_cores=8):
        install_neuronx_cc_hook()
        in_names, out_names, out_avals, zero_outs = [], [], [], []
        for alloc in nc.m.functions[0].allocations:
            if not isinstance(alloc, mybir.MemoryLocationSet):
                continue
            name = alloc.memorylocations[0].name
            if alloc.kind == "ExternalInput":
                in_names.append(name)
            elif alloc.kind == "ExternalOutput":
                out_names.append(name)
                shape = tuple(alloc.tensor_shape)
                dtype = mybir.dt.np(alloc.dtype)
                out_avals.append(jax.core.ShapedArray(shape, dtype))
                zero_outs.append(np.zeros(shape, dtype))
        pname = nc.partition_id_tensor.name if nc.partition_id_tensor else None
        if pname in in_names:
            in_names.remove(pname)
        n_params = len(in_names)
        all_in = in_names + out_names + ([pname] if pname else [])

        def _body(*args):
            ops = list(args)
            if pname:
                ops.append(partition_id_tensor())
            return tuple(_bass_exec_p.bind(
                *ops, out_avals=tuple(out_avals), in_names=tuple(all_in),
                out_names=tuple(out_names), lowering_input_output_aliases=(),
                sim_require_finite=True, sim_require_nnan=True, nc=nc))

        devices = jax.devices()[:n_cores]
        mesh = Mesh(np.asarray(devices), ("core",))
        nio = n_params + len(out_names)
        fn = jax.jit(shard_map(_body, mesh=mesh,
                               in_specs=(PartitionSpec("core"),) * nio,
                               out_specs=(PartitionSpec("core"),) * len(out_names),
                               check_rep=False), keep_unused=True)
        sh = NamedSharding(mesh, PartitionSpec("core"))
        cin = [jax.device_put(np.concatenate(
                   [np.asarray(in_maps[c][nm]) for c in range(n_cores)], axis=0),
                   sh) for nm in in_names]
        czs = [jax.device_put(
                   np.zeros((n_cores * z.shape[0], *z.shape[1:]), z.dtype), sh)
               for z in zero_outs]
        jax.block_until_ready(cin + czs)

        def run():
            jax.block_until_ready(fn(*cin, *czs))
        return run

    mins = {}
    for reps in (1, loop_reps):
        run = runner(_build_nc(reps, hw_loop=(reps > 1)))
        run()
        walls = []
        for _ in range(calls):
            t0 = _time.time()
            run()
            walls.append(_time.time() - t0)
        mins[reps] = min(walls)
    return (mins[loop_reps] - mins[1]) / (loop_reps - 1) * 1e9


# revision 30
# speedup vs baseline: 1.1761x; 1.0148x over previous
"""Batch graph-attention (GAT) layer on 8 TRN2 NeuronCores - Bass/Tile kernel.

kernel(**inputs) takes the FULL inputs
  X [4,2048,64] f32, A [4,2048,2048] f32 (0/1 adjacency),
  W [4,64,64] f32, a_self [4,64] f32, a_neigh [4,64] f32
and returns the FULL output [4,2048,256] f32.

Sharding: data-parallel over (batch, query-half): core c handles batch c//2,
query rows [(c%2)*1024, (c%2)*1024+1024).  No collectives.

Host-side prep is layout-only: per-core slices, X^T / Xq^T transposes,
A slice cast to bf16 (exact for 0/1 adjacency), and the usual weight packing
[W_0..W_3 | W a_self | W a_neigh].  All math runs on device.

Math (per core, per head h), exploiting softmax scale-invariance:
  exp(lrelu_0.2(s1[i]+s2[j])) = exp(s1[i]) * max(E1[j], G[i]*E2[j])
  with E1=exp(s2), E2=exp(0.2*s2), G=exp(-0.8*s1); the exp(s1[i]) factor is
  constant per query column i and cancels in the softmax division, so the
  kernel never materializes it.  Each [128j x 1024i] score tile is then ONE
  dual-scalar TensorScalar on DVE (bf16, 4x mode):
      m = (G_bc * E2[j]) max E1[j]
  followed by one bf16 mask multiply pm = m * A^T (DVE 2x / Pool for a
  subset), and a bf16 feats matmul [lin|1]^T @ pm accumulating numerator +
  denominator in PSUM.  Division (+ ReLU) happens once per (head, query) on
  the Activation engine.

Implementation notes:
 - A^T comes from 16 DMA-xbar transposes ([1024,128] -> [128,1024]) reading
   the bf16 input directly; no staging, no conversion pass.
 - lin / score matmuls run in float32r (1 cycle/row vs 4 for fp32); f32r
   requires producers to round, so X^T / W go through f32r-rounding copies.
 - feats matmuls run in bf16; PSUM accumulates fp32.  Main loop iterates
   j-tile pairs outer / heads inner with 4 concurrent PSUM accumulators, so
   compute consumes A^T tiles in xbar completion order.
 - DMA queues: bulk loads + xbars + output on the SP queue; the
   compute-dependent DMAs (sq scratch write, s_self broadcasts) go on the
   Activation queue so they cannot head-of-line-block the xbars.
 - This walrus build accepts at most one sync-wait per instruction; a
   post-scheduling pass splits Tile's multi-wait instructions into wait-only
   EventSemaphore sequencer ops (engine queues are strict FIFO).
"""
import sys

if "/opt/trn_rl_repo" not in sys.path:
    sys.path.insert(0, "/opt/trn_rl_repo")

import numpy as np
import ml_dtypes
import concourse.bass as bass
import concourse.tile as tile
from concourse import mybir
from concourse.bass_utils import run_bass_kernel_spmd

F32 = mybir.dt.float32
F32R = mybir.dt.float32r
BF16 = mybir.dt.bfloat16

B, N, F, H, FE = 4, 2048, 64, 4, 64
NI = 1024
NT = N // 128
NIC = NI // 128
ALPHA = 0.2
LW = FE + 1
LEXT = H * LW
# (pr*H + h) indices (of 32) whose mask-multiply runs on Pool
POOL_PAIRS = frozenset(
    [pr * H + 3 for pr in range(7)] + [pr * H + 1 for pr in (2, 3, 4)])


def _split_multi_waits(nc, max_waits=1):
    """Split multi-wait instructions (walrus limit: 1 sync-wait per inst)."""
    n_split = 0
    for fn in nc.m.functions:
        for blk in fn.blocks:
            insts = blk.instructions
            i = 0
            while i < len(insts):
                inst = insts[i]
                si = inst.sync_info
                if si is None or len(si.on_wait) <= max_waits:
                    i += 1
                    continue
                waits = list(si.on_wait)
                extra, keep = waits[:-max_waits], waits[-max_waits:]
                for w in extra:
                    ev = mybir.InstEventSemaphore(
                        name=f"{inst.name}_wsplit{n_split}", ins=[], outs=[])
                    ev.engine = inst.engine
                    ev.sync_info = mybir.SyncInfo(on_wait=[w], on_update=[])
                    insts.insert(i, ev)
                    n_split += 1
                    i += 1
                inst.sync_info = mybir.SyncInfo(
                    on_wait=keep, on_update=list(si.on_update))
                i += 1
    return n_split


def _emit(tc, outs, ins, reps=1, hw_loop=False):
    if hw_loop and reps > 1:
        with tc.For_i(0, reps, 1,
                      hint_engines=(mybir.EngineType.PE, mybir.EngineType.DVE,
                                    mybir.EngineType.Activation,
                                    mybir.EngineType.SP,
                                    mybir.EngineType.Pool)):
            _emit_once(tc, outs, ins, 0)
    else:
        for rep in range(reps):
            _emit_once(tc, outs, ins, rep)


def _emit_once(tc, outs, ins, rep):
    """Emit the kernel into an open TileContext."""
    nc = tc.nc
    outD = outs[0] if isinstance(outs, (list, tuple)) else outs
    XTD, AhD, WallD = ins

    const = tc.alloc_tile_pool(name="const", bufs=1)
    persist = tc.alloc_tile_pool(name="persist", bufs=1)
    work = tc.alloc_tile_pool(name="work", bufs=6)
    gwork = tc.alloc_tile_pool(name="gwork", bufs=2)
    outw = tc.alloc_tile_pool(name="outw", bufs=2)
    ps_small = tc.alloc_tile_pool(name="ps_small", bufs=2, space="PSUM")

    # ---- constants / inputs: two packed loads (SP queue) ----
    # WI = [Wall (264 cols, rows 0..63) | Identity (128) | head-selector
    #       one-hot rows (4 x 128 cols, rows 0..3)]
    WI_sb = const.tile([128, LEXT + 4 + 128 + 512], F32)
    nc.sync.dma_start(out=WI_sb, in_=WallD)
    W_sb = WI_sb[0:F, 0:LEXT + 4]
    I_sb = WI_sb[:, LEXT + 4:LEXT + 4 + 128]
    sel_bf = const.tile([H, 512], BF16)
    nc.vector.tensor_copy(sel_bf, WI_sb[0:H, LEXT + 4 + 128:LEXT + 4 + 640])
    # XTT = [XqT (1024 cols) | XT (2048 cols)], rows 0..63
    XTT_sb = persist.tile([F, NI + N], F32)
    nc.sync.dma_start(out=XTT_sb, in_=XTD)
    XqT_sb = XTT_sb[:, 0:NI]
    XT_sb = XTT_sb[:, NI:NI + N]

    # ---- A^T from the bf16 input: 8 pair-granular xbar transposes (SP);
    # 3D out [128, 2, 1024] maps source column s*128+p, row r -> AT[p, s, r]
    AT_p = [persist.tile([128, 2 * NI], BF16, tag=f"ATp{k}", name=f"ATp{k}")
            for k in range(NT // 2)]
    for k in range(NT // 2):
        nc.sync.dma_start_transpose(
            out=AT_p[k].rearrange("p (s f) -> p s f", s=2),
            in_=AhD[:, k * 256:(k + 1) * 256])

    # ---- f32r-rounding copies for the score matmuls ----
    W_r = const.tile([F, LEXT + 4], F32R)
    nc.vector.tensor_copy(W_r, W_sb)
    XqT_r = persist.tile([F, NI], F32R)
    nc.vector.tensor_copy(XqT_r, XqT_sb)
    XT_r = persist.tile([F, N], F32R)
    nc.vector.tensor_copy(XT_r, XT_sb)

    # ---- G = exp(-0.8 * s_self) for this core's queries ----
    # One transposed matmul puts per-head query scores on partitions 0..3:
    # s1T[h, i] = (W a_self)_h . Xq_i; then one exp, one 2KB DRAM write and
    # four contiguous broadcast reads produce the G_bc tiles.
    s1T_ps = ps_small.tile([H, NI], F32, tag="s1T", bufs=1)
    for half in range(2):
        nc.tensor.matmul(
            out=s1T_ps[:, half * 512:(half + 1) * 512],
            lhsT=W_r[:, H * FE:H * FE + H],
            rhs=XqT_r[:, half * 512:(half + 1) * 512],
            start=True, stop=True)
    G_all = persist.tile([H, NI], BF16)
    nc.scalar.activation(out=G_all, in_=s1T_ps, scale=ALPHA - 1.0,
                         func=mybir.ActivationFunctionType.Exp)

    # ---- lin (bf16, for feats matmuls) + neighbor scores s2 ----
    # Separate per-tile tensors so consumers unblock as each tile lands
    # (whole-tile RAW tracking would otherwise chain them all).
    linext = [persist.tile([128, LEXT], BF16, tag=f"lx{t}", name=f"lx{t}")
              for t in range(NT)]
    E1s = [persist.tile([128, 8], F32, tag=f"E1_{t}", name=f"E1_{t}")
           for t in range(NT)]
    E2s = [persist.tile([128, 8], F32, tag=f"E2_{t}", name=f"E2_{t}")
           for t in range(NT)]

    def emit_lin(ts):
        for t in ts:
            lin3 = linext[t].rearrange("p (h c) -> p h c", h=H)
            nc.vector.memset(lin3[:, :, FE:FE + 1], 1.0)
            lin_ps = ps_small.tile([128, LEXT + 4], F32, tag="linps", bufs=2,
                                   name=f"lin_ps{t}")
            nc.tensor.matmul(
                out=lin_ps, lhsT=XT_r[:, t * 128:(t + 1) * 128], rhs=W_r,
                start=True, stop=True)
            nc.scalar.activation(out=E1s[t], in_=lin_ps[:, H * FE:H * FE + 8],
                                 func=mybir.ActivationFunctionType.Exp)
            nc.scalar.activation(out=E2s[t], in_=lin_ps[:, H * FE:H * FE + 8],
                                 scale=ALPHA,
                                 func=mybir.ActivationFunctionType.Exp)
            nc.scalar.copy(
                lin3[:, :, 0:FE],
                lin_ps[:, 0:H * FE].rearrange("p (h o) -> p h o", h=H))

    emit_lin(range(4))

    # G_bc via PE selector broadcasts: sel_h[4,128] (one-hot row h) against
    # G_all[4,NI] -> PSUM [128,NI], copied to bf16 SBUF on Act.  No DMA.
    G_bc = []
    for h in range(H):
        g_ps = ps_small.tile([128, NI], F32, tag="gps", bufs=2,
                             name=f"g_ps{h}")
        for half in range(2):
            nc.tensor.matmul(
                out=g_ps[:, half * 512:(half + 1) * 512],
                lhsT=sel_bf[:, h * 128:(h + 1) * 128],
                rhs=G_all[:, half * 512:(half + 1) * 512],
                start=True, stop=True)
        g = gwork.tile([128, NI], BF16, tag=f"G{h}", name=f"G{h}")
        nc.scalar.copy(g, g_ps)
        G_bc.append(g)
    emit_lin(range(4, NT))

    ps_small.release()
    ps_feats = tc.alloc_tile_pool(name="ps_feats", bufs=1, space="PSUM")

    # ---- main loop: pairs of j-tiles outer (consumes A^T in xbar
    # completion order), heads inner; 4 concurrent PSUM accumulators ----
    feats_ps = [ps_feats.tile([LW, NI], F32, tag=f"feats{h}",
                              name=f"feats{h}") for h in range(H)]
    NPR = NT // 2
    for pr in range(NPR):
        for h in range(H):
            m2 = work.tile([128, 2 * NI], BF16, tag="m2")
            for k in range(2):
                jt = pr * 2 + k
                nc.vector.tensor_scalar(
                    out=m2[:, k * NI:(k + 1) * NI], in0=G_bc[h],
                    scalar1=E2s[jt][:, H + h:H + h + 1],
                    scalar2=E1s[jt][:, H + h:H + h + 1],
                    op0=mybir.AluOpType.mult, op1=mybir.AluOpType.max)
            pm2 = work.tile([128, 2 * NI], BF16, tag="pm2")
            eng = nc.gpsimd if (pr * H + h) in POOL_PAIRS else nc.vector
            eng.tensor_mul(pm2, m2, AT_p[pr])
            for k in range(2):
                for half in range(2):
                    jt = pr * 2 + k
                    nc.tensor.matmul(
                        out=feats_ps[h][:, half * 512:(half + 1) * 512],
                        lhsT=linext[jt][:, h * LW:(h + 1) * LW],
                        rhs=pm2[:, k * NI + half * 512:
                                k * NI + (half + 1) * 512],
                        start=(pr == 0 and k == 0),
                        stop=(pr == NPR - 1 and k == 1))

    # ---- output stage ----
    out_sb = persist.tile([128, NIC * H * FE], F32)
    feats_sb = [outw.tile([LW, NI], F32, tag=f"featsb{h}", name=f"featsb{h}")
                for h in range(H)]
    for h in range(H):
        nc.scalar.copy(feats_sb[h], feats_ps[h])
    ps_feats.release()
    ps_outT = tc.alloc_tile_pool(name="ps_outT", bufs=2, space="PSUM")
    for h in range(H):
        # one [65,128]->[128,65] transpose per query block carries the
        # numerators AND the denominator column; two PSUM tiles of 4 blocks
        # each so no transpose output straddles a PSUM bank boundary
        fT_ps = [ps_outT.tile([128, 4 * LW], F32, tag=f"fT{half}",
                              name=f"fT{half}") for half in range(2)]
        for ic in range(NIC):
            nc.tensor.transpose(
                out=fT_ps[ic // 4][:, (ic % 4) * LW:(ic % 4 + 1) * LW],
                in_=feats_sb[h][:, ic * 128:(ic + 1) * 128],
                identity=I_sb[0:LW, 0:LW])
        recips = outw.tile([128, NIC], F32, tag="recips")
        for half in range(2):
            nc.vector.reciprocal(
                recips[:, half * 4:(half + 1) * 4],
                fT_ps[half].rearrange("p (q c) -> p q c", q=4)[:, :, FE])
        for ic in range(NIC):
            # out = relu(feats / denom), alternating DVE / Act per head
            if h % 2 == 0:
                nc.vector.tensor_scalar(
                    out=out_sb[:, ic * H * FE + h * FE:
                               ic * H * FE + (h + 1) * FE],
                    in0=fT_ps[ic // 4][:, (ic % 4) * LW:(ic % 4) * LW + FE],
                    scalar1=recips[:, ic:ic + 1], scalar2=0.0,
                    op0=mybir.AluOpType.mult, op1=mybir.AluOpType.max)
            else:
                nc.scalar.activation(
                    out=out_sb[:, ic * H * FE + h * FE:
                               ic * H * FE + (h + 1) * FE],
                    in_=fT_ps[ic // 4][:, (ic % 4) * LW:(ic % 4) * LW + FE],
                    scale=recips[:, ic:ic + 1],
                    func=mybir.ActivationFunctionType.Relu)

    nc.sync.dma_start(
        out=outD.rearrange("(t p) o -> p t o", p=128),
        in_=out_sb.rearrange("p (t o) -> p t o", t=NIC))

    for p in (ps_outT, outw, gwork, work, persist, const):
        p.release()


_CACHED = {}


def _build_nc(reps=1, hw_loop=False):
    key = (reps, hw_loop)
    if key in _CACHED:
        return _CACHED[key]
    nc = bass.Bass("TRN2", target_bir_lowering=False, debug=False,
                   num_devices=8)
    xtt = nc.dram_tensor("XTT", [F, NI + N], F32, kind="ExternalInput").ap()
    ah = nc.dram_tensor("Ah", [NI, N], BF16, kind="ExternalInput").ap()
    wi = nc.dram_tensor("WI", [128, LEXT + 4 + 128 + 512], F32,
                        kind="ExternalInput").ap()
    out = nc.dram_tensor("Out", [NI, H * FE], F32, kind="ExternalOutput").ap()
    with tile.TileContext(nc) as tc:
        _emit(tc, [out], [xtt, ah, wi], reps=reps, hw_loop=hw_loop)
    _split_multi_waits(nc)
    _CACHED[key] = nc
    return nc


def _make_in_maps(X, A, W, a_self, a_neigh):
    C2self = np.einsum("hfo,ho->fh", W, a_self)
    C2neigh = np.einsum("hfo,ho->fh", W, a_neigh)
    Wall = np.concatenate(
        [W[h] for h in range(H)] + [C2self, C2neigh], axis=1)
    WI = np.zeros((128, LEXT + 4 + 128 + 512), np.float32)
    WI[0:F, 0:LEXT + 4] = Wall
    WI[:, LEXT + 4:LEXT + 4 + 128] = np.eye(128, dtype=np.float32)
    for h in range(H):
        WI[h, LEXT + 4 + 128 + h * 128:LEXT + 4 + 128 + (h + 1) * 128] = 1.0
    in_maps = []
    for c in range(8):
        b, ih = c // 2, c % 2
        i0 = ih * NI
        XTT = np.concatenate(
            [X[b, i0:i0 + NI].T, X[b].T], axis=1).astype(np.float32)
        in_maps.append({
            "XTT": np.ascontiguousarray(XTT),
            "Ah": np.ascontiguousarray(
                A[b, i0:i0 + NI, :]).astype(ml_dtypes.bfloat16),
            "WI": WI,
        })
    return in_maps


def kernel(X, A, W, a_self, a_neigh):
    X = np.asarray(X, np.float32)
    A = np.asarray(A, np.float32)
    W = np.asarray(W, np.float32)
    a_self = np.asarray(a_self, np.float32)
    a_neigh = np.asarray(a_neigh, np.float32)
    in_maps = _make_in_maps(X, A, W, a_self, a_neigh)
    nc = _build_nc()
    res = run_bass_kernel_spmd(nc, in_maps, list(range(8)))
    out = np.empty((B, N, H * FE), np.float32)
    for c in range(8):
        b, ih = c // 2, c % 2
        out[b, ih * NI:(ih + 1) * NI, :] = res.results[c]["Out"]
    return out


def measure_exec_ns(inputs, loop_reps=512, calls=8):
    """Differential device-time measurement: wrap the kernel body in an
    on-device For_i loop with `loop_reps` iterations; with device-resident
    inputs, exec_ns = (min_wall(loop) - min_wall(single)) / (loop_reps - 1).
    Each iteration re-reads all inputs from HBM (full single-shot kernel,
    with a full inter-iteration barrier at the loop back-edge)."""
    import time as _time
    import jax
    from jax.sharding import Mesh, PartitionSpec, NamedSharding
    from jax.experimental.shard_map import shard_map
    from concourse.bass2jax import (_bass_exec_p, install_neuronx_cc_hook,
                                    partition_id_tensor)

    in_maps = _make_in_maps(
        np.asarray(inputs["X"], np.float32), np.asarray(inputs["A"], np.float32),
        np.asarray(inputs["W"], np.float32),
        np.asarray(inputs["a_self"], np.float32),
        np.asarray(inputs["a_neigh"], np.float32))

    def runner(nc, n_cores=8):
        install_neuronx_cc_hook()
        in_names, out_names, out_avals, zero_outs = [], [], [], []
        for alloc in nc.m.functions[0].allocations:
            if not isinstance(alloc, mybir.MemoryLocationSet):
                continue
            name = alloc.memorylocations[0].name
            if alloc.kind == "ExternalInput":
                in_names.append(name)
            elif alloc.kind == "ExternalOutput":
                out_names.append(name)
                shape = tuple(alloc.tensor_shape)
                dtype = mybir.dt.np(alloc.dtype)
                out_avals.append(jax.core.ShapedArray(shape, dtype))
                zero_outs.append(np.zeros(shape, dtype))
        pname = nc.partition_id_tensor.name if nc.partition_id_tensor else None
        if pname in in_names:
            in_names.remove(pname)
        n_params = len(in_names)
        all_in = in_names + out_names + ([pname] if pname else [])

        def _body(*args):
            ops = list(args)
            if pname:
                ops.append(partition_id_tensor())
            return tuple(_bass_exec_p.bind(
                *ops, out_avals=tuple(out_avals), in_names=tuple(all_in),
                out_names=tuple(out_names), lowering_input_output_aliases=(),
                sim_require_finite=True, sim_require_nnan=True, nc=nc))

        devices = jax.devices()[:n_cores]
        mesh = Mesh(np.asarray(devices), ("core",))
        nio = n_params + len(out_names)
        fn = jax.jit(shard_map(_body, mesh=mesh,
                               in_specs=(PartitionSpec("core"),) * nio,
                               out_specs=(PartitionSpec("core"),) * len(out_names),
                               check_rep=False), keep_unused=True)
        sh = NamedSharding(mesh, PartitionSpec("core"))
        cin = [jax.device_put(np.concatenate(
                   [np.asarray(in_maps[c][nm]) for c in range(n_cores)], axis=0),
                   sh) for nm in in_names]
        czs = [jax.device_put(
                   np.zeros((n_cores * z.shape[0], *z.shape[1:]), z.dtype), sh)
               for z in zero_outs]
        jax.block_until_ready(cin + czs)

        def run():
            jax.block_until_ready(fn(*cin, *czs))
        return run

    mins = {}
    for reps in (1, loop_reps):
        run = runner(_build_nc(reps, hw_loop=(reps > 1)))
        run()
        walls = []
        for _ in range(calls):
            t0 = _time.time()
            run()
            walls.append(_time.time() - t0)
        mins[reps] = min(walls)
    return (mins[loop_reps] - mins[1]) / (loop_reps - 1) * 1e9
